# Initial kernel scaffold
#
"""CBConv2d (change-based conv) Trainium2 kernel, 8-core SPMD.

Reference semantics (B=1, C=64, H=W=512, 3x3 SAME conv):
  changed = any_c(|inp - prev_input| > 0.1)            # [H, W]
  dilated = maxpool3x3(changed)                        # [H, W]
  out     = dilated ? (conv2d(inp, w) + bias) : prev_output

Sharding: H split across 8 cores (64 rows each), halos materialized on host.

Per-core device pipeline (4 tiles of 16 output rows):
  - inputs arrive as bf16 (host pre-cast); conv runs on TensorE in bf16 with
    fp32 PSUM accumulation, rows paired (r, r+8) across partition halves so
    every epilogue op runs on 128 partitions.
  - change mask: DVE subtract, ACT Square, ACT Relu(x - thr^2) -> "ind";
    per-pixel change count AND the H-dilation come from 18 matmuls with
    3-wide banded ones weights; W-dilation is 2 small DVE adds; PE ones-
    matmuls broadcast the dilated count across partitions into PSUM; one
    copy_predicated merges conv over prev_output.

Mask exactness note: inputs are bf16-rounded, so pixels whose |diff| sits
within ~0.4% of the threshold can flip vs the fp32 reference. A flipped
pixel only affects the output if its entire 3x3 neighborhood has no other
changed pixel; with this data distribution (~95% changed) the expected
number of affected output pixels is ~1e-7.
"""
import numpy as np
import ml_dtypes

import concourse.bass as bass
import concourse.mybir as mybir
import concourse.tile as tile
from concourse import bacc
from concourse.bass_utils import run_bass_kernel_spmd

F32 = mybir.dt.float32
BF16 = mybir.dt.bfloat16
BF = ml_dtypes.bfloat16

C = 64          # channels
H = W = 512     # spatial
NCORES = 8
RPC = H // NCORES          # rows per core (64)
R = 16                     # output rows per tile
NT = RPC // R              # tiles per core (4)
NPAD = R + 2               # padded rows per tile (18)
G = 10                     # rows per partition-group (overlapping: lower=0..9, upper=8..17)
WP = W + 2                 # padded width (514)
THR2 = float(np.float32(0.1) * np.float32(0.1))

_cached = {}


def build_nc(loop_iters: int = 0, variant: str = "full"):
    """Build the per-core Bass program. loop_iters>0 wraps the whole pipeline
    in a For_i loop that re-executes it (for slope-based timing).

    variant tokens (comma-joined) progressively strip stages for debugging:
      nosel   - plain copy instead of copy_predicated
      intmask - materialize uint8 mask via tensor_scalar instead of bitcast
      nomb    - also skip mask-broadcast matmuls
      nodil   - also skip W-dilation + dil1 DMA
      nocnt   - also skip count matmuls
      noind   - also skip indicator ops (pure conv kernel)
      noconv  - skip conv matmuls + evac (mask pipeline only; copy prev->out)
      nob2    - unpacked 18-MM conv straight from xt (no doubled buffers)
    """
    has_ind = "noind" not in variant
    has_cnt = has_ind and "nocnt" not in variant
    has_dil = has_cnt and "nodil" not in variant
    has_mb = has_dil and "nomb" not in variant
    has_sel = has_mb and "nosel" not in variant and "intmask" not in variant
    has_intmask = has_mb and "intmask" in variant
    has_conv = "noconv" not in variant
    has_b2 = "nob2" not in variant

    nc = bacc.Bacc("TRN2", target_bir_lowering=False, debug=False,
                   enable_asserts=True, num_devices=NCORES)

    xin = nc.dram_tensor("xin", [NT, 128, G * WP], BF16, kind="ExternalInput")
    pin = nc.dram_tensor("pin", [NT, 128, G * WP], BF16, kind="ExternalInput")
    pout = nc.dram_tensor("pout", [NT, 128, 8 * W], F32, kind="ExternalInput")
    wt = nc.dram_tensor("wt", [128, 9 * 64], BF16, kind="ExternalInput")
    sel = nc.dram_tensor("sel", [128, G * R], BF16, kind="ExternalInput")
    ones64 = nc.dram_tensor("ones64", [1, 64], BF16, kind="ExternalInput")
    wt2 = nc.dram_tensor("wt2", [128, 3 * 64], BF16, kind="ExternalInput")
    sel2x = nc.dram_tensor("sel2x", [2, 128], BF16, kind="ExternalInput")
    biasv = nc.dram_tensor("biasv", [128, 1], F32, kind="ExternalInput")
    wtbd = nc.dram_tensor("wtbd", [128, 9 * 128], BF16, kind="ExternalInput")
    outd = nc.dram_tensor("out", [NT, 128, 8 * W], F32, kind="ExternalOutput")

    with tile.TileContext(nc) as tc:
        with tc.tile_pool(name="consts", bufs=1) as cpool, \
             tc.tile_pool(name="io", bufs=2) as iopool, \
             tc.tile_pool(name="mask", bufs=2) as mpool, \
             tc.tile_pool(name="cnt", bufs=2, space="PSUM") as cntpool, \
             tc.tile_pool(name="conv", bufs=4, space="PSUM") as convpool, \
             tc.tile_pool(name="mb", bufs=2, space="PSUM") as mbpool:

            wtt = cpool.tile([128, 9 * 64], BF16)
            selt = cpool.tile([128, G * R], BF16)
            onest = cpool.tile([1, 64], BF16)
            wt2t = cpool.tile([128, 3 * 64], BF16)
            sel2xt = cpool.tile([2, 128], BF16)
            biast = cpool.tile([128, 1], F32)
            wtbdt = cpool.tile([128, 9 * 128], BF16)
            negthr = cpool.tile([128, 1], F32)
            nc.sync.dma_start(out=wtt[:], in_=wt[:])
            nc.sync.dma_start(out=selt[:], in_=sel[:])
            nc.sync.dma_start(out=onest[:], in_=ones64[:])
            nc.sync.dma_start(out=wt2t[:], in_=wt2[:])
            nc.sync.dma_start(out=sel2xt[:], in_=sel2x[:])
            nc.sync.dma_start(out=biast[:], in_=biasv[:])
            nc.sync.dma_start(out=wtbdt[:], in_=wtbd[:])
            nc.vector.memset(negthr[:], -THR2)

            def xtap(buf, g, k, dw):
                return buf[64 * g:64 * (g + 1), k * WP + dw:k * WP + dw + W]

            def emit_tile(t):
                xt = iopool.tile([128, G * WP], BF16, tag="xt")
                pt = iopool.tile([128, G * WP], BF16, tag="pt")
                pvt = iopool.tile([128, 8 * W], F32, tag="pvt")
                nc.sync.dma_start(out=xt[:], in_=xin[t])
                nc.sync.dma_start(out=pt[:], in_=pin[t])
                nc.sync.dma_start(out=pvt[:], in_=pout[t])


                dil1 = None
                if has_ind:
                    # --- change indicator: relu((x - p)^2 - thr^2), bf16 ---
                    ind = mpool.tile([128, G * WP], BF16, tag="ind")
                    nc.vector.tensor_tensor(out=ind[:], in0=xt[:], in1=pt[:],
                                            op=mybir.AluOpType.subtract)
                    nc.scalar.activation(ind[:], ind[:],
                                         mybir.ActivationFunctionType.Square)
                    nc.scalar.activation(ind[:], ind[:],
                                         mybir.ActivationFunctionType.Relu,
                                         bias=negthr[:])

                if has_cnt:
                    # --- change count + H-dilation via banded matmuls ---
                    # one 128-deep MM per slot k contracts group0 row k
                    # (partitions 0:64, band(k)) AND group1 row k+8
                    # (partitions 64:128, band(k+8)) -- rows 8,9 get counted
                    # twice, which is harmless: only nonzero-ness is used.
                    cnt = cntpool.tile([R, W], F32, tag="cnt")
                    for k in range(G):
                        nc.tensor.matmul(
                            cnt[:],
                            selt[:, k * R:(k + 1) * R],
                            ind[:, k * WP + 1:k * WP + 1 + W],
                            start=(k == 0), stop=(k == G - 1))

                if has_dil:
                    # --- W-dilation on [R, W+2] ---
                    hs = mpool.tile([R, WP], F32, tag="hs")
                    nc.vector.memset(hs[:], 0.0)
                    nc.vector.tensor_copy(out=hs[:, 1:W + 1], in_=cnt[:])
                    t1 = mpool.tile([R, W + 1], F32, tag="t1")
                    nc.vector.tensor_tensor(out=t1[:], in0=hs[:, 0:W + 1],
                                            in1=hs[:, 1:WP],
                                            op=mybir.AluOpType.add)
                    dil = mpool.tile([R, W], BF16, tag="dil")
                    nc.vector.tensor_tensor(out=dil[:], in0=t1[:, 0:W],
                                            in1=hs[:, 2:WP],
                                            op=mybir.AluOpType.add)
                    dil1 = mpool.tile([2, 8 * W], BF16, tag="dil1")
                    nc.scalar.dma_start(out=dil1[:], in_=dil[:])

                conv_sb = iopool.tile([128, 8 * W], F32, tag="conv_sb")
                cbs = []
                for j in range(8):
                    sl = slice(j * W, (j + 1) * W)
                    if has_conv:
                        # --- conv pair: rows (j, j+8) -> one PSUM bank ---
                        cb = convpool.tile([128, W], F32, tag="cb")
                        # block-diag lhsT [[W,0],[0,W]] computes BOTH halves
                        # of the pair in one 128-contraction MM: partitions
                        # 0:64 = group0 row j+dh -> out 0:64; partitions
                        # 64:128 = group1 row j+dh (= row 8+j+dh) -> 64:128.
                        taps = [(dh, dw) for dh in range(3) for dw in range(3)]
                        for i, (dh, dw) in enumerate(taps):
                            ti = dh * 3 + dw
                            nc.tensor.matmul(
                                cb[:],
                                wtbdt[:, ti * 128:(ti + 1) * 128],
                                xt[:, (j + dh) * WP + dw:
                                   (j + dh) * WP + dw + W],
                                start=(i == 0), stop=(i == len(taps) - 1))
                        # --- evacuate conv + bias ---
                        nc.scalar.activation(
                            conv_sb[:, sl], cb[:],
                            mybir.ActivationFunctionType.Identity,
                            bias=biast[:])

                for j in range(8):
                    sl = slice(j * W, (j + 1) * W)
                    if has_mb:
                        # --- broadcast dilated counts for rows (j, j+8) ---
                        mb = mbpool.tile([128, W], F32, tag="mb")
                        nc.tensor.matmul(mb[:], sel2xt[:],
                                         dil1[:, j * W:(j + 1) * W],
                                         start=True, stop=True)

                    # --- merge conv over prev_output ---
                    if has_sel and has_conv:
                        nc.vector.copy_predicated(
                            pvt[:, sl], mb[:].bitcast(mybir.dt.int32),
                            conv_sb[:, sl])
                    elif has_intmask and has_conv:
                        mi = mpool.tile([128, W], mybir.dt.uint8, tag="mi")
                        nc.vector.tensor_scalar(out=mi[:], in0=mb[:],
                                                scalar1=0.0, scalar2=None,
                                                op0=mybir.AluOpType.is_gt)
                        nc.vector.copy_predicated(pvt[:, sl], mi[:],
                                                  conv_sb[:, sl])
                    elif has_conv:
                        nc.vector.tensor_copy(out=pvt[:, sl],
                                              in_=conv_sb[:, sl])

                nc.scalar.dma_start(out=outd[t], in_=pvt[:])

            if loop_iters > 0:
                with tc.For_i(0, loop_iters, 1,
                              hint_engines=(mybir.EngineType.PE,
                                            mybir.EngineType.DVE,
                                            mybir.EngineType.Activation,
                                            mybir.EngineType.SP)):
                    for t in range(NT):
                        emit_tile(t)
            else:
                for t in range(NT):
                    emit_tile(t)

    nc.compile()
    return nc


def host_prep(inp, prev_input, prev_output, weight, bias):
    """Build per-core in_maps."""
    inp = np.asarray(inp)
    prev_input = np.asarray(prev_input)
    prev_output = np.asarray(prev_output)
    weight = np.asarray(weight)
    bias = np.asarray(bias)

    xpad = np.zeros((C, H + 2, WP), dtype=BF)
    ppad = np.zeros((C, H + 2, WP), dtype=BF)
    xpad[:, 1:H + 1, 1:W + 1] = inp[0].astype(BF)
    ppad[:, 1:H + 1, 1:W + 1] = prev_input[0].astype(BF)

    # weights: wt[ci + 64g, (dh*3+dw)*64 + co] = weight[co, ci, dh, dw]
    wtap = weight.transpose(1, 2, 3, 0).reshape(C, 9 * C).astype(BF)
    wt = np.concatenate([wtap, wtap], axis=0)  # [128, 576]

    # sel bands: group0 handles padded rows p=0..9 (k=p), group1 p=10..17
    # (k=p-8). sel[ci + 64g, k*R + u] = 1 if p-2 <= u <= p, 0 <= u < R.
    selA = np.zeros((G, R), dtype=BF)
    selB = np.zeros((G, R), dtype=BF)
    for p in range(0, 10):
        for u in range(max(0, p - 2), min(R, p + 1)):
            selA[p, u] = 1
    for p in range(10, NPAD):
        for u in range(max(0, p - 2), min(R, p + 1)):
            selB[p - 8, u] = 1
    sel = np.empty((128, G * R), dtype=BF)
    sel[:64] = selA.reshape(1, G * R)
    sel[64:] = selB.reshape(1, G * R)

    ones64 = np.ones((1, 64), dtype=BF)
    # packed 2-tap weights: rows 0:64 = tap (dh=0,dw), rows 64:128 = (dh=1,dw)
    wt2 = np.empty((128, 3 * 64), dtype=BF)
    for dw in range(3):
        wt2[:64, dw * 64:(dw + 1) * 64] = weight[:, :, 0, dw].T.astype(BF)
        wt2[64:, dw * 64:(dw + 1) * 64] = weight[:, :, 1, dw].T.astype(BF)
    wtbd = np.zeros((128, 9 * 128), dtype=BF)
    for dh in range(3):
        for dw in range(3):
            ti = dh * 3 + dw
            wtap_ = weight[:, :, dh, dw].T.astype(BF)   # [ci, co]
            wtbd[0:64, ti * 128:ti * 128 + 64] = wtap_
            wtbd[64:128, ti * 128 + 64:(ti + 1) * 128] = wtap_

    sel2x = np.zeros((2, 128), dtype=BF)
    sel2x[0, :64] = 1
    sel2x[1, 64:] = 1
    biasv = np.tile(bias.astype(np.float32).reshape(-1, 1), (2, 1))  # [128,1]

    in_maps = []
    for c in range(NCORES):
        r0 = c * RPC

        def slab(pad):
            s = np.empty((NT, 128, G * WP), dtype=BF)
            for t in range(NT):
                rows = pad[:, r0 + 16 * t: r0 + 16 * t + NPAD, :]  # [C,18,WP]
                s[t, :64] = rows[:, 0:10].reshape(C, G * WP)
                s[t, 64:] = rows[:, 8:18].reshape(C, G * WP)
            return s

        po = prev_output[0][:, r0:r0 + RPC, :].reshape(C, NT, 2, 8, W)
        po = po.transpose(1, 2, 0, 3, 4).reshape(NT, 128, 8 * W)
        po = np.ascontiguousarray(po, dtype=np.float32)

        in_maps.append({
            "xin": slab(xpad), "pin": slab(ppad), "pout": po,
            "wt": wt, "sel": sel, "ones64": ones64, "biasv": biasv,
            "wtbd": wtbd,
            "wt2": wt2, "sel2x": sel2x,
        })
    return in_maps


def host_post(results):
    """Reassemble [NCORES] x [NT, 128, 8*W] -> [1, C, H, W] fp32."""
    out = np.empty((1, C, H, W), dtype=np.float32)
    for c, res in enumerate(results):
        o = res["out"].reshape(NT, 2, C, 8, W).transpose(2, 0, 1, 3, 4)
        out[0, :, c * RPC:(c + 1) * RPC, :] = o.reshape(C, RPC, W)
    return out


def kernel(inp, prev_input, prev_output, weight, bias):
    if "nc" not in _cached:
        _cached["nc"] = build_nc(0)
    nc = _cached["nc"]
    in_maps = host_prep(inp, prev_input, prev_output, weight, bias)
    res = run_bass_kernel_spmd(nc, in_maps, core_ids=list(range(NCORES)))
    return host_post(res.results)


if __name__ == "__main__":
    rng = np.random.default_rng(0)
    inp = rng.standard_normal((1, C, H, W), dtype=np.float32)
    prev_input = inp + 0.05 * rng.standard_normal((1, C, H, W), dtype=np.float32)
    prev_output = rng.standard_normal((1, C, H, W), dtype=np.float32)
    weight = (0.05 * rng.standard_normal((C, C, 3, 3))).astype(np.float32)
    bias = rng.standard_normal(C).astype(np.float32)
    out = kernel(inp=inp, prev_input=prev_input, prev_output=prev_output,
                 weight=weight, bias=bias)
    print("out", out.shape, out.dtype, float(np.abs(out).mean()))



# revision 1
# speedup vs baseline: 3.3996x; 3.3996x over previous
"""CBConv2d (change-based conv) Trainium2 kernel, 8-core SPMD.

Reference semantics (B=1, C=64, H=W=512, 3x3 SAME conv):
  changed = any_c(|inp - prev_input| > 0.1)            # [H, W]
  dilated = maxpool3x3(changed)                        # [H, W]
  out     = dilated ? (conv2d(inp, w) + bias) : prev_output

Sharding: H split across 8 cores (64 rows each), halos materialized on host.

Per-core device pipeline (4 tiles of 16 output rows):
  - inputs arrive as bf16 (host pre-cast); conv runs on TensorE in bf16 with
    fp32 PSUM accumulation, rows paired (r, r+8) across partition halves so
    every epilogue op runs on 128 partitions.
  - change mask: DVE subtract, ACT Square, ACT Relu(x - thr^2) -> "ind";
    per-pixel change count AND the H-dilation come from 18 matmuls with
    3-wide banded ones weights; W-dilation is 2 small DVE adds; PE ones-
    matmuls broadcast the dilated count across partitions into PSUM; one
    copy_predicated merges conv over prev_output.

Mask exactness note: inputs are bf16-rounded, so pixels whose |diff| sits
within ~0.4% of the threshold can flip vs the fp32 reference. A flipped
pixel only affects the output if its entire 3x3 neighborhood has no other
changed pixel; with this data distribution (~95% changed) the expected
number of affected output pixels is ~1e-7.
"""
import numpy as np
import ml_dtypes

import concourse.bass as bass
import concourse.mybir as mybir
import concourse.tile as tile
from concourse import bacc
from concourse.bass_utils import run_bass_kernel_spmd

F32 = mybir.dt.float32
BF16 = mybir.dt.bfloat16
BF = ml_dtypes.bfloat16

C = 64          # channels
H = W = 512     # spatial
NCORES = 8
RPC = H // NCORES          # rows per core (64)
R = 16                     # output rows per tile
NT = RPC // R              # tiles per core (4)
NPAD = R + 2               # padded rows per tile (18)
G = 10                     # rows per partition-group (overlapping: lower=0..9, upper=8..17)
WP = W + 2                 # padded width (514)
THR2 = float(np.float32(0.1) * np.float32(0.1))

_cached = {}


def build_nc(loop_iters: int = 0, variant: str = "full"):
    """Build the per-core Bass program. loop_iters>0 wraps the whole pipeline
    in a For_i loop that re-executes it (for slope-based timing).

    variant tokens (comma-joined) progressively strip stages for debugging:
      nosel   - plain copy instead of copy_predicated
      intmask - materialize uint8 mask via tensor_scalar instead of bitcast
      nomb    - also skip mask-broadcast matmuls
      nodil   - also skip W-dilation + dil1 DMA
      nocnt   - also skip count matmuls
      noind   - also skip indicator ops (pure conv kernel)
      noconv  - skip conv matmuls + evac (mask pipeline only; copy prev->out)
      nob2    - unpacked 18-MM conv straight from xt (no doubled buffers)
    """
    has_ind = "noind" not in variant
    has_cnt = has_ind and "nocnt" not in variant
    has_dil = has_cnt and "nodil" not in variant
    has_mb = has_dil and "nomb" not in variant
    has_sel = has_mb and "nosel" not in variant and "intmask" not in variant
    has_intmask = has_mb and "intmask" in variant
    has_conv = "noconv" not in variant
    has_b2 = "nob2" not in variant

    nc = bacc.Bacc("TRN2", target_bir_lowering=False, debug=False,
                   enable_asserts=True, num_devices=NCORES)

    xin = nc.dram_tensor("xin", [NT, 128, G * WP], BF16, kind="ExternalInput")
    pin = nc.dram_tensor("pin", [NT, 128, G * WP], BF16, kind="ExternalInput")
    pout = nc.dram_tensor("pout", [NT, 128, 8 * W], F32, kind="ExternalInput")
    wt = nc.dram_tensor("wt", [128, 9 * 64], BF16, kind="ExternalInput")
    sel = nc.dram_tensor("sel", [128, G * R], BF16, kind="ExternalInput")
    ones64 = nc.dram_tensor("ones64", [1, 64], BF16, kind="ExternalInput")
    wt2 = nc.dram_tensor("wt2", [128, 3 * 64], BF16, kind="ExternalInput")
    sel2x = nc.dram_tensor("sel2x", [2, 128], BF16, kind="ExternalInput")
    biasv = nc.dram_tensor("biasv", [128, 1], F32, kind="ExternalInput")
    wtbd = nc.dram_tensor("wtbd", [128, 9 * 128], BF16, kind="ExternalInput")
    outd = nc.dram_tensor("out", [NT, 128, 8 * W], F32, kind="ExternalOutput")

    with tile.TileContext(nc) as tc:
        with tc.tile_pool(name="consts", bufs=1) as cpool, \
             tc.tile_pool(name="io", bufs=2) as iopool, \
             tc.tile_pool(name="mask", bufs=2) as mpool, \
             tc.tile_pool(name="cnt", bufs=2, space="PSUM") as cntpool, \
             tc.tile_pool(name="conv", bufs=4, space="PSUM") as convpool, \
             tc.tile_pool(name="mb", bufs=2, space="PSUM") as mbpool:

            wtt = cpool.tile([128, 9 * 64], BF16)
            selt = cpool.tile([128, G * R], BF16)
            onest = cpool.tile([1, 64], BF16)
            wt2t = cpool.tile([128, 3 * 64], BF16)
            sel2xt = cpool.tile([2, 128], BF16)
            biast = cpool.tile([128, 1], F32)
            wtbdt = cpool.tile([128, 9 * 128], BF16)
            negthr = cpool.tile([128, 1], F32)
            nc.sync.dma_start(out=wtt[:], in_=wt[:])
            nc.sync.dma_start(out=selt[:], in_=sel[:])
            nc.sync.dma_start(out=onest[:], in_=ones64[:])
            nc.sync.dma_start(out=wt2t[:], in_=wt2[:])
            nc.sync.dma_start(out=sel2xt[:], in_=sel2x[:])
            nc.sync.dma_start(out=biast[:], in_=biasv[:])
            nc.sync.dma_start(out=wtbdt[:], in_=wtbd[:])
            nc.vector.memset(negthr[:], -THR2)

            def xtap(buf, g, k, dw):
                return buf[64 * g:64 * (g + 1), k * WP + dw:k * WP + dw + W]

            def emit_tile(t):
                xt = iopool.tile([128, G * WP], BF16, tag="xt")
                pt = iopool.tile([128, G * WP], BF16, tag="pt")
                pvt = iopool.tile([128, 8 * W], F32, tag="pvt")
                nc.sync.dma_start(out=xt[:], in_=xin[t])
                nc.sync.dma_start(out=pt[:], in_=pin[t])
                nc.sync.dma_start(out=pvt[:], in_=pout[t])


                dil1 = None
                if has_ind:
                    # --- change indicator: relu((x - p)^2 - thr^2), bf16 ---
                    ind = mpool.tile([128, G * WP], BF16, tag="ind")
                    nc.vector.tensor_tensor(out=ind[:], in0=xt[:], in1=pt[:],
                                            op=mybir.AluOpType.subtract)
                    nc.scalar.activation(ind[:], ind[:],
                                         mybir.ActivationFunctionType.Square)
                    nc.scalar.activation(ind[:], ind[:],
                                         mybir.ActivationFunctionType.Relu,
                                         bias=negthr[:])

                if has_cnt:
                    # --- change count + H-dilation via banded matmuls ---
                    # one 128-deep MM per slot k contracts group0 row k
                    # (partitions 0:64, band(k)) AND group1 row k+8
                    # (partitions 64:128, band(k+8)) -- rows 8,9 get counted
                    # twice, which is harmless: only nonzero-ness is used.
                    cnt = cntpool.tile([R, W], F32, tag="cnt")
                    for k in range(G):
                        nc.tensor.matmul(
                            cnt[:],
                            selt[:, k * R:(k + 1) * R],
                            ind[:, k * WP + 1:k * WP + 1 + W],
                            start=(k == 0), stop=(k == G - 1))

                if has_dil:
                    # --- W-dilation on [R, W+2] ---
                    hs = mpool.tile([R, WP], F32, tag="hs")
                    nc.vector.memset(hs[:], 0.0)
                    nc.vector.tensor_copy(out=hs[:, 1:W + 1], in_=cnt[:])
                    t1 = mpool.tile([R, W + 1], F32, tag="t1")
                    nc.vector.tensor_tensor(out=t1[:], in0=hs[:, 0:W + 1],
                                            in1=hs[:, 1:WP],
                                            op=mybir.AluOpType.add)
                    dil = mpool.tile([R, W], BF16, tag="dil")
                    nc.vector.tensor_tensor(out=dil[:], in0=t1[:, 0:W],
                                            in1=hs[:, 2:WP],
                                            op=mybir.AluOpType.add)
                    dil1 = mpool.tile([2, 8 * W], BF16, tag="dil1")
                    nc.scalar.dma_start(out=dil1[:], in_=dil[:])

                conv_sb = iopool.tile([128, 8 * W], F32, tag="conv_sb")
                cbs = []
                for j in range(8):
                    sl = slice(j * W, (j + 1) * W)
                    if has_conv:
                        # --- conv pair: rows (j, j+8) -> one PSUM bank ---
                        cb = convpool.tile([128, W], F32, tag="cb")
                        # block-diag lhsT [[W,0],[0,W]] computes BOTH halves
                        # of the pair in one 128-contraction MM: partitions
                        # 0:64 = group0 row j+dh -> out 0:64; partitions
                        # 64:128 = group1 row j+dh (= row 8+j+dh) -> 64:128.
                        taps = [(dh, dw) for dh in range(3) for dw in range(3)]
                        for i, (dh, dw) in enumerate(taps):
                            ti = dh * 3 + dw
                            nc.tensor.matmul(
                                cb[:],
                                wtbdt[:, ti * 128:(ti + 1) * 128],
                                xt[:, (j + dh) * WP + dw:
                                   (j + dh) * WP + dw + W],
                                start=(i == 0), stop=(i == len(taps) - 1))
                        # --- evacuate conv + bias ---
                        nc.scalar.activation(
                            conv_sb[:, sl], cb[:],
                            mybir.ActivationFunctionType.Identity,
                            bias=biast[:])

                for j in range(8):
                    sl = slice(j * W, (j + 1) * W)
                    if has_mb:
                        # --- broadcast dilated counts for rows (j, j+8) ---
                        mb = mbpool.tile([128, W], F32, tag="mb")
                        nc.tensor.matmul(mb[:], sel2xt[:],
                                         dil1[:, j * W:(j + 1) * W],
                                         start=True, stop=True)

                    # --- merge conv over prev_output ---
                    if has_sel and has_conv:
                        nc.vector.copy_predicated(
                            pvt[:, sl], mb[:].bitcast(mybir.dt.int32),
                            conv_sb[:, sl])
                    elif has_intmask and has_conv:
                        mi = mpool.tile([128, W], mybir.dt.uint8, tag="mi")
                        nc.vector.tensor_scalar(out=mi[:], in0=mb[:],
                                                scalar1=0.0, scalar2=None,
                                                op0=mybir.AluOpType.is_gt)
                        nc.vector.copy_predicated(pvt[:, sl], mi[:],
                                                  conv_sb[:, sl])
                    elif has_conv:
                        nc.vector.tensor_copy(out=pvt[:, sl],
                                              in_=conv_sb[:, sl])

                nc.scalar.dma_start(out=outd[t], in_=pvt[:])

            if loop_iters > 0:
                with tc.For_i(0, loop_iters, 1,
                              hint_engines=(mybir.EngineType.PE,
                                            mybir.EngineType.DVE,
                                            mybir.EngineType.Activation,
                                            mybir.EngineType.SP)):
                    for t in range(NT):
                        emit_tile(t)
            else:
                for t in range(NT):
                    emit_tile(t)

    nc.compile()
    return nc


def host_prep(inp, prev_input, prev_output, weight, bias):
    """Build per-core in_maps."""
    inp = np.asarray(inp)
    prev_input = np.asarray(prev_input)
    prev_output = np.asarray(prev_output)
    weight = np.asarray(weight)
    bias = np.asarray(bias)

    xpad = np.zeros((C, H + 2, WP), dtype=BF)
    ppad = np.zeros((C, H + 2, WP), dtype=BF)
    xpad[:, 1:H + 1, 1:W + 1] = inp[0].astype(BF)
    ppad[:, 1:H + 1, 1:W + 1] = prev_input[0].astype(BF)

    # weights: wt[ci + 64g, (dh*3+dw)*64 + co] = weight[co, ci, dh, dw]
    wtap = weight.transpose(1, 2, 3, 0).reshape(C, 9 * C).astype(BF)
    wt = np.concatenate([wtap, wtap], axis=0)  # [128, 576]

    # sel bands: group0 handles padded rows p=0..9 (k=p), group1 p=10..17
    # (k=p-8). sel[ci + 64g, k*R + u] = 1 if p-2 <= u <= p, 0 <= u < R.
    selA = np.zeros((G, R), dtype=BF)
    selB = np.zeros((G, R), dtype=BF)
    for p in range(0, 10):
        for u in range(max(0, p - 2), min(R, p + 1)):
            selA[p, u] = 1
    for p in range(10, NPAD):
        for u in range(max(0, p - 2), min(R, p + 1)):
            selB[p - 8, u] = 1
    sel = np.empty((128, G * R), dtype=BF)
    sel[:64] = selA.reshape(1, G * R)
    sel[64:] = selB.reshape(1, G * R)

    ones64 = np.ones((1, 64), dtype=BF)
    # packed 2-tap weights: rows 0:64 = tap (dh=0,dw), rows 64:128 = (dh=1,dw)
    wt2 = np.empty((128, 3 * 64), dtype=BF)
    for dw in range(3):
        wt2[:64, dw * 64:(dw + 1) * 64] = weight[:, :, 0, dw].T.astype(BF)
        wt2[64:, dw * 64:(dw + 1) * 64] = weight[:, :, 1, dw].T.astype(BF)
    wtbd = np.zeros((128, 9 * 128), dtype=BF)
    for dh in range(3):
        for dw in range(3):
            ti = dh * 3 + dw
            wtap_ = weight[:, :, dh, dw].T.astype(BF)   # [ci, co]
            wtbd[0:64, ti * 128:ti * 128 + 64] = wtap_
            wtbd[64:128, ti * 128 + 64:(ti + 1) * 128] = wtap_

    sel2x = np.zeros((2, 128), dtype=BF)
    sel2x[0, :64] = 1
    sel2x[1, 64:] = 1
    biasv = np.tile(bias.astype(np.float32).reshape(-1, 1), (2, 1))  # [128,1]

    in_maps = []
    for c in range(NCORES):
        r0 = c * RPC

        def slab(pad):
            s = np.empty((NT, 128, G * WP), dtype=BF)
            for t in range(NT):
                rows = pad[:, r0 + 16 * t: r0 + 16 * t + NPAD, :]  # [C,18,WP]
                s[t, :64] = rows[:, 0:10].reshape(C, G * WP)
                s[t, 64:] = rows[:, 8:18].reshape(C, G * WP)
            return s

        po = prev_output[0][:, r0:r0 + RPC, :].reshape(C, NT, 2, 8, W)
        po = po.transpose(1, 2, 0, 3, 4).reshape(NT, 128, 8 * W)
        po = np.ascontiguousarray(po, dtype=np.float32)

        in_maps.append({
            "xin": slab(xpad), "pin": slab(ppad), "pout": po,
            "wt": wt, "sel": sel, "ones64": ones64, "biasv": biasv,
            "wtbd": wtbd,
            "wt2": wt2, "sel2x": sel2x,
        })
    return in_maps


def host_post(results):
    """Reassemble [NCORES] x [NT, 128, 8*W] -> [1, C, H, W] fp32."""
    out = np.empty((1, C, H, W), dtype=np.float32)
    for c, res in enumerate(results):
        o = res["out"].reshape(NT, 2, C, 8, W).transpose(2, 0, 1, 3, 4)
        out[0, :, c * RPC:(c + 1) * RPC, :] = o.reshape(C, RPC, W)
    return out


def kernel(inp, prev_input, prev_output, weight, bias):
    if "nc" not in _cached:
        _cached["nc"] = build_nc(0)
    nc = _cached["nc"]
    in_maps = host_prep(inp, prev_input, prev_output, weight, bias)
    res = run_bass_kernel_spmd(nc, in_maps, core_ids=list(range(NCORES)))
    return host_post(res.results)


if __name__ == "__main__":
    rng = np.random.default_rng(0)
    inp = rng.standard_normal((1, C, H, W), dtype=np.float32)
    prev_input = inp + 0.05 * rng.standard_normal((1, C, H, W), dtype=np.float32)
    prev_output = rng.standard_normal((1, C, H, W), dtype=np.float32)
    weight = (0.05 * rng.standard_normal((C, C, 3, 3))).astype(np.float32)
    bias = rng.standard_normal(C).astype(np.float32)
    out = kernel(inp=inp, prev_input=prev_input, prev_output=prev_output,
                 weight=weight, bias=bias)
    print("out", out.shape, out.dtype, float(np.abs(out).mean()))

